# revision 21
# baseline (speedup 1.0000x reference)
"""Multi-head attention (B=4, N=2048, C=1024, H=16, Dh=64) on 8 TRN2 NeuronCores.

Sharding: tensor-parallel over heads — core c owns heads (2c, 2c+1) for all
batches.  Each core computes its 2 heads' QKV projection, attention, and the
partial output projection (contraction over its 128 head-dims of w_proj);
the host sums the 8 partial projections (bf16) and adds the bias.

Per-core pipeline (unit = one batch of 2048 tokens):
  - host passes xT = x^T [1024, 8192] so channels land on SBUF partitions
  - QT/KT/VT computed as [128(d, 2 heads stacked), t] tiles
  - scores computed TRANSPOSED: ST[k, q] = KT_h.T @ QT_h (contraction d=64,
    two heads row-packed into the PE array: h0 rows 0-63, h1 rows 64-127)
  - softmax without max-subtraction (scores verified: |s|*scale < 10):
    ACT exp reads the score PSUM pair [128, 1024] directly, writes PT
  - AV: O^T[d, q]; VSB row layout [V_h0 | 1 | 1 | V_h1] so h0 accumulates
    into PSUM rows 0:65 (denominator row 64) and h1 into rows 63:128
    (denominator row 63) — both normalize halves are partition-aligned
    vector ops, no cross-partition DMA
  - proj: out[t, o] = OT_tile.T @ wpT, stored bf16 (host sums in f32)

Emission is a flat software-pipelined slot stream over (unit, q-span,
k-chunk): each slot emits [scores(g), exp(g), AV(g-1), filler(g-1)] so the
scalar engine is fed before the AV that waits on the previous exp, and
fillers (next unit's prefetch/qkv/transposes, previous block's norm+proj)
never head-of-line block the attention chain.  x-tile DMAs prefetch a full
unit ahead.
"""

import os
import numpy as np

B, N, C = 4, 2048, 1024
H, Dh = 16, 64
NT = B * N            # 8192 tokens
NCORES = 8
HPC = H // NCORES     # 2 heads per core
SCALE = Dh ** -0.5

TPU = N               # tokens per unit (one batch)
QS = 512              # q-span
KC = 128              # k-chunk
MHA_DTYPE = os.environ.get("MHA_DTYPE", "bf16")

_CACHE = {}


def _np_in_dtype():
    if MHA_DTYPE == "bf16":
        import ml_dtypes
        return np.dtype(ml_dtypes.bfloat16)
    return np.dtype(np.float32)


def _build_program():
    import concourse.bacc as bacc
    import concourse.bass as bass
    import concourse.tile as tile
    from concourse import mybir
    from concourse.masks import make_identity

    f32 = mybir.dt.float32
    din = {
        "bf16": mybir.dt.bfloat16,
        "f32r": mybir.dt.float32r,
        "f32": mybir.dt.float32,
    }[MHA_DTYPE]

    nc = bacc.Bacc("TRN2", target_bir_lowering=False, debug=False)

    xT = nc.dram_tensor("xT", [C, NT], din, kind="ExternalInput").ap()
    wqkvT = nc.dram_tensor("wqkvT", [C, 6 * Dh], din, kind="ExternalInput").ap()
    wpT = nc.dram_tensor("wpT", [2 * Dh, C], din, kind="ExternalInput").ap()
    out = nc.dram_tensor("out", [NT, C], din, kind="ExternalOutput").ap()

    NCC = C // 128        # 8 c-chunks
    NTT = TPU // QS       # 4 t-tiles per unit
    NKC = TPU // KC       # 16 k-chunks per unit
    NQS = TPU // QS       # 4 q-spans per unit
    KPT = QS // KC        # 4 k-chunks per t-tile
    VW = 2 * (Dh + 1)     # 130: V_sb row layout [V_h0 | 1 | 1 | V_h1]
    SPU = NQS * NKC       # 64 slots per unit

    with tile.TileContext(nc) as tc:
        with (
            tc.tile_pool(name="const", bufs=1) as const,
            tc.tile_pool(name="xp", bufs=36) as xp,
            tc.tile_pool(name="qt", bufs=2) as qtp,
            tc.tile_pool(name="kt", bufs=2) as ktp,
            tc.tile_pool(name="vt", bufs=2) as vtp,
            tc.tile_pool(name="vsb", bufs=2) as vsbp,
            tc.tile_pool(name="pt", bufs=4) as ptp,
            tc.tile_pool(name="ot", bufs=2) as otp,
            tc.tile_pool(name="rn", bufs=2) as rnp,
            tc.tile_pool(name="po", bufs=4) as pop,
            tc.tile_pool(name="mps", bufs=2, space="PSUM") as mps,
            tc.tile_pool(name="stps", bufs=2, space="PSUM") as stps,
            tc.tile_pool(name="avps", bufs=1, space="PSUM") as avps,
        ):
            # per-chunk weight tiles: the first projection matmul only waits
            # on its own 98KB DMA, not the whole weight load
            wq_sb = []
            for cc in range(NCC):
                w = const.tile([128, 6 * Dh], din, name=f"wq{cc}")
                nc.scalar.dma_start(out=w, in_=wqkvT[cc * 128:(cc + 1) * 128, :])
                wq_sb.append(w)
            ones_row = const.tile([1, Dh], f32, name="ones_row")
            nc.vector.memset(ones_row, 1.0)

            # per-unit persistent tiles, allocated lazily
            QT, KT, VT, VSB, OT = {}, {}, {}, {}, {}
            XS = {}               # (u, tt) -> list of 8 prefetched x tiles

            def alloc_unit(u):
                QT[u] = qtp.tile([128, TPU], din, tag="QT", name=f"QT{u}")
                KT[u] = ktp.tile([128, TPU], din, tag="KT", name=f"KT{u}")
                VT[u] = vtp.tile([128, TPU], din, tag="VT", name=f"VT{u}")
                VSB[u] = vsbp.tile([128, NKC * VW], din, tag="VSB", name=f"VSB{u}")

            def prefetch_item(u, tt):
                """Issue the 8 x-tile DMAs for t-tile (u, tt)."""
                def run():
                    if tt == 0:
                        alloc_unit(u)
                    t0 = u * TPU
                    cells = []
                    for cc in range(NCC):
                        xt = xp.tile([128, QS], din, tag="xs", name="xt")
                        eng = nc.sync if cc % 2 == 0 else nc.gpsimd
                        eng.dma_start(
                            out=xt,
                            in_=xT[cc * 128:(cc + 1) * 128,
                                   t0 + tt * QS:t0 + (tt + 1) * QS],
                        )
                        cells.append(xt)
                    XS[(u, tt)] = cells
                return run

            def qkv_group_item(u, tt, grp):
                """Projection group grp (0=Q,1=K,2=V) for t-tile (u, tt)."""
                def run():
                    xs = XS[(u, tt)]
                    ps = mps.tile([128, QS], f32, tag="m", name="ps")
                    for cc in range(NCC):
                        w_sl = wq_sb[cc][:, grp * 128:(grp + 1) * 128]
                        nc.tensor.matmul(
                            ps, w_sl, xs[cc],
                            start=(cc == 0), stop=(cc == NCC - 1),
                            skip_group_check=True,
                        )
                    tgt = (QT, KT, VT)[grp][u]
                    nc.vector.tensor_copy(
                        tgt[:, tt * QS:(tt + 1) * QS], ps)
                    if grp == 2:
                        del XS[(u, tt)]
                return run

            def transpose_item(u, tt):
                """V transposes (4 k-chunks) for t-tile (u, tt) -> VSB."""
                def run():
                    for j in range(KPT):
                        kc = tt * KPT + j
                        tp = mps.tile([128, 128], din, tag="m", name="tp")
                        nc.tensor.transpose(
                            tp, VT[u][:, kc * 128:(kc + 1) * 128], ident)
                        base = kc * VW
                        nc.vector.tensor_copy(
                            VSB[u][:, base: base + Dh], tp[:, 0:Dh])
                        nc.vector.memset(
                            VSB[u][:, base + Dh: base + Dh + 1], 1.0)
                        nc.vector.tensor_copy(
                            VSB[u][:, base + Dh + 1: base + 2 * Dh + 1],
                            tp[:, Dh: 2 * Dh])
                        nc.vector.memset(
                            VSB[u][:, base + 2 * Dh + 1: base + VW], 1.0)
                return run

            # ---- slot scheduler ----
            slot_items = {}       # g -> fillers, run after AV of slot g
            pre_items = {}        # g -> items run before scores of slot g

            def at_slot(d, g, item):
                d.setdefault(g, []).append(item)

            def run_due(g):
                for it in slot_items.pop(g, ()):
                    it()

            def drain_upto(g):
                for gg in sorted(k for k in slot_items if k <= g):
                    for it in slot_items.pop(gg, ()):
                        it()

            def evict_oh(oh, flush=False):
                """Evict AV accumulators to SBUF (frees PSUM) and start the
                reciprocal/broadcast chain so the normalize muls scheduled a
                slot later find their inputs ready."""
                osbs, Rbs = [], []
                for i in range(2):
                    osb = rnp.tile([Dh + 1, QS], f32, tag=f"osb{i}",
                                   name=f"osb{i}")
                    nc.vector.tensor_copy(osb, oh[i])
                    osbs.append(osb)
                for i in range(2):
                    d_row = rnp.tile([1, QS], f32, tag=f"d{i}", name="d_row")
                    nc.vector.tensor_copy(d_row, osbs[i][Dh:Dh + 1, :])
                    r_row = rnp.tile([1, QS], f32, tag=f"r{i}", name="r_row")
                    nc.vector.reciprocal_approx_fast(r_row, d_row)
                    if flush:
                        # tensor engine is idle during the flush: broadcast
                        # via outer product (saves 2us of gpsimd latency)
                        Rb = mps.tile([Dh, QS], f32, tag="m", name="Rb")
                        nc.tensor.matmul(Rb, ones_row, r_row,
                                         skip_group_check=True)
                    else:
                        Rb = rnp.tile([Dh, QS], f32, tag=f"R{i}", name="Rb")
                        nc.gpsimd.partition_broadcast(Rb, r_row)
                    Rbs.append(Rb)
                return osbs, Rbs

            def normalize_items(u, qs, osbs, Rbs):
                if qs == 0:
                    OT[u] = otp.tile([128, TPU], din, tag="OT",
                                     name=f"OT{u}")
                q0 = qs * QS

                def norm(i):
                    def run():
                        if i == 0:
                            nc.vector.tensor_mul(
                                OT[u][0:Dh, q0:q0 + QS], osbs[0][0:Dh, :],
                                Rbs[0])
                        else:
                            tmp = rnp.tile([Dh, QS], din, tag="tmp",
                                           name="tmp")
                            nc.vector.tensor_mul(tmp, osbs[1][0:Dh, :], Rbs[1])
                            nc.sync.dma_start(
                                out=OT[u][Dh:128, q0:q0 + QS], in_=tmp)
                    return run

                return [norm(0), norm(1)]

            def proj_items(u, qs):
                t0 = u * TPU

                def proj_pair(tt):
                    def run():
                        for osp in range(C // QS):
                            pp = mps.tile([128, QS], f32, tag="m", name="pp")
                            nc.tensor.matmul(
                                pp,
                                OT[u][:, tt * 128:(tt + 1) * 128],
                                wp_sb[:, osp * QS:(osp + 1) * QS],
                                skip_group_check=True,
                            )
                            po = pop.tile([128, QS], din, name="po")
                            nc.vector.tensor_copy(po, pp)
                            eng = nc.sync if (tt + osp) % 2 == 0 else nc.gpsimd
                            eng.dma_start(
                                out=out[t0 + tt * 128: t0 + (tt + 1) * 128,
                                        osp * QS:(osp + 1) * QS],
                                in_=po,
                            )
                    return run

                return [proj_pair(qs * (QS // 128) + tl)
                        for tl in range(QS // 128)]

            # ---- emission: flat software-pipelined slot stream ----
            # startup: unit 0 x-tiles prefetch immediately (sync queue runs
            # concurrently with the weight loads on the gpsimd queue)
            pf0 = [prefetch_item(0, tt) for tt in range(NTT)]
            pf0[0]()
            pf0[1]()

            ident = const.tile([128, 128], din)
            make_identity(nc, ident)
            wp_sb = const.tile([128, C], din)
            nc.gpsimd.dma_start(out=wp_sb, in_=wpT)
            pf0[2]()
            pf0[3]()

            # unit 0's qkv: inline right before the first slot that needs it
            for tt in range(NTT):
                for grp in range(3):
                    at_slot(pre_items, 4 * tt, qkv_group_item(0, tt, grp))
                at_slot(pre_items, 4 * tt, transpose_item(0, tt))

            # units 1..3: prefetch + qkv spread over unit u-1's slots
            for nu in range(1, B):
                g0 = (nu - 1) * SPU
                for tt in range(NTT):
                    at_slot(slot_items, g0 + 1 + 4 * tt, prefetch_item(nu, tt))
                hard = []
                for tt in range(NTT):
                    for grp in range(3):
                        hard.append(qkv_group_item(nu, tt, grp))
                    hard.append(transpose_item(nu, tt))
                for i, it in enumerate(hard):
                    at_slot(slot_items, g0 + 16 + 3 * i, it)

            pend = None           # (g, u, kc, oh, pt) awaiting AV emission
            cur_oh = None

            def emit_av(p):
                g, u, kc, oh, pt = p
                for i in range(2):
                    vbase = kc * VW + i * (Dh + 1)
                    nc.tensor.matmul(
                        oh[i],
                        VSB[u][:, vbase: vbase + Dh + 1],
                        pt[:, i * QS:(i + 1) * QS],
                        start=(kc == 0), stop=(kc == NKC - 1),
                        skip_group_check=True,
                    )
                run_due(g)
                if kc == NKC - 1:
                    qs = (g % SPU) // NKC
                    osbs, Rbs = evict_oh(oh, flush=(g == B * SPU - 1))
                    softs = (normalize_items(u, qs, osbs, Rbs)
                             + proj_items(u, qs))
                    if g == B * SPU - 1:
                        for it in softs:      # final flush
                            it()
                    else:
                        # norm muls early; projs late enough that the OT
                        # writes (incl. the h1 DMA) have cleared
                        for i, it in enumerate(softs[:2]):
                            at_slot(slot_items, g + 2 + i, it)
                        for i, it in enumerate(softs[2:]):
                            at_slot(slot_items, g + 8 + 2 * i, it)

            for u in range(B):
                for qs in range(NQS):
                    q0 = qs * QS
                    for kc in range(NKC):
                        g = u * SPU + qs * NKC + kc
                        for it in pre_items.pop(g, ()):
                            it()
                        sp = stps.tile([128, 2 * QS], f32, name="sp")
                        nc.tensor.matmul(
                            sp[:, 0:QS],
                            KT[u][0:Dh, kc * 128:(kc + 1) * 128],
                            QT[u][0:Dh, q0:q0 + QS],
                            skip_group_check=True,
                        )
                        nc.tensor.matmul(
                            sp[:, QS:2 * QS],
                            KT[u][Dh:128, kc * 128:(kc + 1) * 128],
                            QT[u][Dh:128, q0:q0 + QS],
                            skip_group_check=True,
                        )
                        pt = ptp.tile([128, 2 * QS], din, name="pt")
                        nc.scalar.activation(
                            pt, sp, mybir.ActivationFunctionType.Exp,
                            scale=SCALE,
                        )
                        if pend is not None:
                            emit_av(pend)
                        if kc == 0:
                            cur_oh = [
                                avps.tile([Dh + 1, QS], f32, tag="av0",
                                          name="oh0"),
                                avps.tile([Dh + 1, QS], f32, tag="av1",
                                          name="oh1"),
                            ]
                        pend = (g, u, kc, cur_oh, pt)

            emit_av(pend)
            drain_upto(B * SPU)

    nc.compile()
    return nc


def _shard_inputs(x, w_qkv, w_proj):
    dt = _np_in_dtype()
    xT = np.ascontiguousarray(x.reshape(NT, C).T).astype(dt)
    in_maps = []
    for c in range(NCORES):
        h0, h1 = HPC * c, HPC * c + 1
        rows = []
        for grp in range(3):          # q, k, v
            for h in (h0, h1):
                rows.append(w_qkv[grp * C + h * Dh: grp * C + (h + 1) * Dh])
        wqkvT_c = np.ascontiguousarray(np.concatenate(rows, 0).T).astype(dt)
        wpT_c = np.ascontiguousarray(
            w_proj[:, 2 * Dh * c: 2 * Dh * (c + 1)].T).astype(dt)
        in_maps.append({"xT": xT, "wqkvT": wqkvT_c, "wpT": wpT_c})
    return in_maps


def kernel(x, w_qkv, w_proj, b_proj, _trace=False, _tmpdir=None):
    from concourse import bass_utils

    if "nc" not in _CACHE:
        _CACHE["nc"] = _build_program()
    nc = _CACHE["nc"]

    in_maps = _shard_inputs(
        np.asarray(x, np.float32),
        np.asarray(w_qkv, np.float32),
        np.asarray(w_proj, np.float32),
    )
    res = bass_utils.run_bass_kernel_spmd(
        nc, in_maps, core_ids=list(range(NCORES)),
        trace=_trace, tmpdir=_tmpdir,
    )
    total = res.results[0]["out"].astype(np.float32)
    for c in range(1, NCORES):
        total += res.results[c]["out"].astype(np.float32)
    total += np.asarray(b_proj, np.float32)[None, :]
    out = total.reshape(B, N, C)
    if _trace:
        return out, res
    return out


# revision 24
# speedup vs baseline: 1.0303x; 1.0303x over previous
"""Multi-head attention (B=4, N=2048, C=1024, H=16, Dh=64) on 8 TRN2 NeuronCores.

Sharding: tensor-parallel over heads — core c owns heads (2c, 2c+1) for all
batches.  Each core computes its 2 heads' QKV projection, attention, and the
partial output projection (contraction over its 128 head-dims of w_proj);
the host sums the 8 partial projections (bf16) and adds the bias.

Per-core pipeline (unit = one batch of 2048 tokens):
  - host passes xT = x^T [1024, 8192] so channels land on SBUF partitions
  - QT/KT/VT computed as [128(d, 2 heads stacked), t] tiles
  - scores computed TRANSPOSED: ST[k, q] = KT_h.T @ QT_h (contraction d=64,
    two heads row-packed into the PE array: h0 rows 0-63, h1 rows 64-127)
  - softmax without max-subtraction (scores verified: |s|*scale < 10):
    ACT exp reads the score PSUM pair [128, 1024] directly, writes PT
  - AV: O^T[d, q]; VSB row layout [V_h0 | 1 | 1 | V_h1] so h0 accumulates
    into PSUM rows 0:65 (denominator row 64) and h1 into rows 63:128
    (denominator row 63) — both normalize halves are partition-aligned
    vector ops, no cross-partition DMA
  - proj: out[t, o] = OT_tile.T @ wpT, stored bf16 (host sums in f32)

Emission is a flat software-pipelined slot stream over (unit, q-span,
k-chunk): each slot emits [scores(g), exp(g), AV(g-1), filler(g-1)] so the
scalar engine is fed before the AV that waits on the previous exp, and
fillers (next unit's prefetch/qkv/transposes, previous block's norm+proj)
never head-of-line block the attention chain.  x-tile DMAs prefetch a full
unit ahead.
"""

import os
import numpy as np

B, N, C = 4, 2048, 1024
H, Dh = 16, 64
NT = B * N            # 8192 tokens
NCORES = 8
HPC = H // NCORES     # 2 heads per core
SCALE = Dh ** -0.5

TPU = N               # tokens per unit (one batch)
QS = 512              # q-span
KC = 128              # k-chunk
MHA_DTYPE = os.environ.get("MHA_DTYPE", "bf16")

_CACHE = {}


def _np_in_dtype():
    if MHA_DTYPE == "bf16":
        import ml_dtypes
        return np.dtype(ml_dtypes.bfloat16)
    return np.dtype(np.float32)


def _build_program():
    import concourse.bacc as bacc
    import concourse.bass as bass
    import concourse.tile as tile
    from concourse import mybir
    from concourse.masks import make_identity

    f32 = mybir.dt.float32
    din = {
        "bf16": mybir.dt.bfloat16,
        "f32r": mybir.dt.float32r,
        "f32": mybir.dt.float32,
    }[MHA_DTYPE]

    nc = bacc.Bacc("TRN2", target_bir_lowering=False, debug=False)

    xT = nc.dram_tensor("xT", [C, NT], din, kind="ExternalInput").ap()
    wqkvT = nc.dram_tensor("wqkvT", [C, 6 * Dh], din, kind="ExternalInput").ap()
    wpT = nc.dram_tensor("wpT", [2 * Dh, C], din, kind="ExternalInput").ap()
    out = nc.dram_tensor("out", [NT, C], din, kind="ExternalOutput").ap()

    NCC = C // 128        # 8 c-chunks
    NTT = TPU // QS       # 4 t-tiles per unit
    NKC = TPU // KC       # 16 k-chunks per unit
    NQS = TPU // QS       # 4 q-spans per unit
    KPT = QS // KC        # 4 k-chunks per t-tile
    VW = 2 * (Dh + 1)     # 130: V_sb row layout [V_h0 | 1 | 1 | V_h1]
    SPU = NQS * NKC       # 64 slots per unit

    with tile.TileContext(nc) as tc:
        with (
            tc.tile_pool(name="const", bufs=1) as const,
            tc.tile_pool(name="xp", bufs=36) as xp,
            tc.tile_pool(name="qt", bufs=2) as qtp,
            tc.tile_pool(name="kt", bufs=2) as ktp,
            tc.tile_pool(name="vt", bufs=2) as vtp,
            tc.tile_pool(name="vsb", bufs=2) as vsbp,
            tc.tile_pool(name="pt", bufs=4) as ptp,
            tc.tile_pool(name="ot", bufs=2) as otp,
            tc.tile_pool(name="rn", bufs=2) as rnp,
            tc.tile_pool(name="po", bufs=4) as pop,
            tc.tile_pool(name="mps", bufs=2, space="PSUM") as mps,
            tc.tile_pool(name="stps", bufs=2, space="PSUM") as stps,
            tc.tile_pool(name="avps", bufs=1, space="PSUM") as avps,
        ):
            # per-chunk weight tiles: the first projection matmul only waits
            # on its own 98KB DMA, not the whole weight load
            wq_sb = []
            for cc in range(NCC):
                w = const.tile([128, 6 * Dh], din, name=f"wq{cc}")
                nc.scalar.dma_start(out=w, in_=wqkvT[cc * 128:(cc + 1) * 128, :])
                wq_sb.append(w)
            ones_row = const.tile([1, Dh], f32, name="ones_row")
            nc.vector.memset(ones_row, 1.0)

            # per-unit persistent tiles, allocated lazily
            QT, KT, VT, VSB, OT = {}, {}, {}, {}, {}
            XS = {}               # (u, tt) -> list of 8 prefetched x tiles

            def alloc_unit(u):
                QT[u] = qtp.tile([128, TPU], din, tag="QT", name=f"QT{u}")
                KT[u] = ktp.tile([128, TPU], din, tag="KT", name=f"KT{u}")
                VT[u] = vtp.tile([128, TPU], din, tag="VT", name=f"VT{u}")
                VSB[u] = vsbp.tile([128, NKC * VW], din, tag="VSB", name=f"VSB{u}")

            def prefetch_item(u, tt):
                """Issue the 8 x-tile DMAs for t-tile (u, tt)."""
                def run():
                    if tt == 0:
                        alloc_unit(u)
                    t0 = u * TPU
                    cells = []
                    for cc in range(NCC):
                        xt = xp.tile([128, QS], din, tag="xs", name="xt")
                        eng = nc.sync
                        eng.dma_start(
                            out=xt,
                            in_=xT[cc * 128:(cc + 1) * 128,
                                   t0 + tt * QS:t0 + (tt + 1) * QS],
                        )
                        cells.append(xt)
                    XS[(u, tt)] = cells
                return run

            def qkv_group_item(u, tt, grp):
                """Projection group grp (0=Q,1=K,2=V) for t-tile (u, tt)."""
                def run():
                    xs = XS[(u, tt)]
                    ps = mps.tile([128, QS], f32, tag="m", name="ps")
                    for cc in range(NCC):
                        w_sl = wq_sb[cc][:, grp * 128:(grp + 1) * 128]
                        nc.tensor.matmul(
                            ps, w_sl, xs[cc],
                            start=(cc == 0), stop=(cc == NCC - 1),
                            skip_group_check=True,
                        )
                    tgt = (QT, KT, VT)[grp][u]
                    nc.vector.tensor_copy(
                        tgt[:, tt * QS:(tt + 1) * QS], ps)
                    if grp == 2:
                        del XS[(u, tt)]
                return run

            def transpose_item(u, tt):
                """V transposes (4 k-chunks) for t-tile (u, tt) -> VSB."""
                def run():
                    for j in range(KPT):
                        kc = tt * KPT + j
                        tp = mps.tile([128, 128], din, tag="m", name="tp")
                        nc.tensor.transpose(
                            tp, VT[u][:, kc * 128:(kc + 1) * 128], ident)
                        base = kc * VW
                        nc.vector.tensor_copy(
                            VSB[u][:, base: base + Dh], tp[:, 0:Dh])
                        nc.vector.memset(
                            VSB[u][:, base + Dh: base + Dh + 1], 1.0)
                        nc.vector.tensor_copy(
                            VSB[u][:, base + Dh + 1: base + 2 * Dh + 1],
                            tp[:, Dh: 2 * Dh])
                        nc.vector.memset(
                            VSB[u][:, base + 2 * Dh + 1: base + VW], 1.0)
                return run

            # ---- slot scheduler ----
            slot_items = {}       # g -> fillers, run after AV of slot g
            pre_items = {}        # g -> items run before scores of slot g

            def at_slot(d, g, item):
                d.setdefault(g, []).append(item)

            def run_due(g):
                for it in slot_items.pop(g, ()):
                    it()

            def drain_upto(g):
                for gg in sorted(k for k in slot_items if k <= g):
                    for it in slot_items.pop(gg, ()):
                        it()

            def evict_oh(oh, flush=False):
                """Evict AV accumulators to SBUF (frees PSUM) and start the
                reciprocal/broadcast chain so the normalize muls scheduled a
                slot later find their inputs ready."""
                osbs, Rbs = [], []
                for i in range(2):
                    osb = rnp.tile([Dh + 1, QS], f32, tag=f"osb{i}",
                                   name=f"osb{i}")
                    nc.vector.tensor_copy(osb, oh[i])
                    osbs.append(osb)
                for i in range(2):
                    d_row = rnp.tile([1, QS], f32, tag=f"d{i}", name="d_row")
                    nc.vector.tensor_copy(d_row, osbs[i][Dh:Dh + 1, :])
                    r_row = rnp.tile([1, QS], f32, tag=f"r{i}", name="r_row")
                    nc.vector.reciprocal_approx_fast(r_row, d_row)
                    if flush:
                        # tensor engine is idle during the flush: broadcast
                        # via outer product (saves 2us of gpsimd latency)
                        Rb = mps.tile([Dh, QS], f32, tag="m", name="Rb")
                        nc.tensor.matmul(Rb, ones_row, r_row,
                                         skip_group_check=True)
                    else:
                        Rb = rnp.tile([Dh, QS], f32, tag=f"R{i}", name="Rb")
                        nc.gpsimd.partition_broadcast(Rb, r_row)
                    Rbs.append(Rb)
                return osbs, Rbs

            def normalize_items(u, qs, osbs, Rbs):
                if qs == 0:
                    OT[u] = otp.tile([128, TPU], din, tag="OT",
                                     name=f"OT{u}")
                q0 = qs * QS

                def norm(i):
                    def run():
                        if i == 0:
                            nc.vector.tensor_mul(
                                OT[u][0:Dh, q0:q0 + QS], osbs[0][0:Dh, :],
                                Rbs[0])
                        else:
                            tmp = rnp.tile([Dh, QS], din, tag="tmp",
                                           name="tmp")
                            nc.vector.tensor_mul(tmp, osbs[1][0:Dh, :], Rbs[1])
                            nc.sync.dma_start(
                                out=OT[u][Dh:128, q0:q0 + QS], in_=tmp)
                    return run

                return [norm(0), norm(1)]

            def proj_items(u, qs, flush=False):
                t0 = u * TPU

                def proj_pair(tt):
                    def run():
                        for osp in range(C // QS):
                            pp = mps.tile([128, QS], f32, tag="m", name="pp")
                            nc.tensor.matmul(
                                pp,
                                OT[u][:, tt * 128:(tt + 1) * 128],
                                wp_sb[:, osp * QS:(osp + 1) * QS],
                                skip_group_check=True,
                            )
                            po = pop.tile([128, QS], din, name="po")
                            nc.vector.tensor_copy(po, pp)
                            # flush: spread the final stores across the two
                            # idle DMA-capable queues to shorten the drain
                            eng = (nc.scalar if flush and (tt + osp) % 2
                                   else nc.sync)
                            eng.dma_start(
                                out=out[t0 + tt * 128: t0 + (tt + 1) * 128,
                                        osp * QS:(osp + 1) * QS],
                                in_=po,
                            )
                    return run

                return [proj_pair(qs * (QS // 128) + tl)
                        for tl in range(QS // 128)]

            # ---- emission: flat software-pipelined slot stream ----
            # startup: unit 0 x-tiles prefetch immediately (sync queue runs
            # concurrently with the weight loads on the gpsimd queue)
            pf0 = [prefetch_item(0, tt) for tt in range(NTT)]
            pf0[0]()
            pf0[1]()

            ident = const.tile([128, 128], din)
            make_identity(nc, ident)
            wp_sb = const.tile([128, C], din)
            nc.gpsimd.dma_start(out=wp_sb, in_=wpT)
            pf0[2]()
            pf0[3]()

            # unit 0's qkv: inline right before the first slot that needs it
            for tt in range(NTT):
                for grp in range(3):
                    at_slot(pre_items, 4 * tt, qkv_group_item(0, tt, grp))
                at_slot(pre_items, 4 * tt, transpose_item(0, tt))

            # units 1..3: prefetch + qkv spread over unit u-1's slots
            for nu in range(1, B):
                g0 = (nu - 1) * SPU
                for tt in range(NTT):
                    at_slot(slot_items, g0 + 1 + 4 * tt, prefetch_item(nu, tt))
                hard = []
                for tt in range(NTT):
                    for grp in range(3):
                        hard.append(qkv_group_item(nu, tt, grp))
                    hard.append(transpose_item(nu, tt))
                for i, it in enumerate(hard):
                    at_slot(slot_items, g0 + 16 + 3 * i, it)

            pend = None           # (g, u, kc, oh, pt) awaiting AV emission
            cur_oh = None

            def emit_av(p):
                g, u, kc, oh, pt = p
                for i in range(2):
                    vbase = kc * VW + i * (Dh + 1)
                    nc.tensor.matmul(
                        oh[i],
                        VSB[u][:, vbase: vbase + Dh + 1],
                        pt[:, i * QS:(i + 1) * QS],
                        start=(kc == 0), stop=(kc == NKC - 1),
                        skip_group_check=True,
                    )
                run_due(g)
                if kc == NKC - 1:
                    qs = (g % SPU) // NKC
                    fl = g == B * SPU - 1
                    osbs, Rbs = evict_oh(oh, flush=fl)
                    softs = (normalize_items(u, qs, osbs, Rbs)
                             + proj_items(u, qs, flush=fl))
                    if g == B * SPU - 1:
                        for it in softs:      # final flush
                            it()
                    else:
                        # norm muls early; projs late enough that the OT
                        # writes (incl. the h1 DMA) have cleared
                        for i, it in enumerate(softs[:2]):
                            at_slot(slot_items, g + 2 + i, it)
                        for i, it in enumerate(softs[2:]):
                            at_slot(slot_items, g + 8 + 2 * i, it)

            for u in range(B):
                for qs in range(NQS):
                    q0 = qs * QS
                    for kc in range(NKC):
                        g = u * SPU + qs * NKC + kc
                        for it in pre_items.pop(g, ()):
                            it()
                        sp = stps.tile([128, 2 * QS], f32, name="sp")
                        nc.tensor.matmul(
                            sp[:, 0:QS],
                            KT[u][0:Dh, kc * 128:(kc + 1) * 128],
                            QT[u][0:Dh, q0:q0 + QS],
                            skip_group_check=True,
                        )
                        nc.tensor.matmul(
                            sp[:, QS:2 * QS],
                            KT[u][Dh:128, kc * 128:(kc + 1) * 128],
                            QT[u][Dh:128, q0:q0 + QS],
                            skip_group_check=True,
                        )
                        pt = ptp.tile([128, 2 * QS], din, name="pt")
                        nc.scalar.activation(
                            pt, sp, mybir.ActivationFunctionType.Exp,
                            scale=SCALE,
                        )
                        if pend is not None:
                            emit_av(pend)
                        if kc == 0:
                            cur_oh = [
                                avps.tile([Dh + 1, QS], f32, tag="av0",
                                          name="oh0"),
                                avps.tile([Dh + 1, QS], f32, tag="av1",
                                          name="oh1"),
                            ]
                        pend = (g, u, kc, cur_oh, pt)

            emit_av(pend)
            drain_upto(B * SPU)

    nc.compile()
    return nc


def _shard_inputs(x, w_qkv, w_proj):
    dt = _np_in_dtype()
    xT = np.ascontiguousarray(x.reshape(NT, C).T).astype(dt)
    in_maps = []
    for c in range(NCORES):
        h0, h1 = HPC * c, HPC * c + 1
        rows = []
        for grp in range(3):          # q, k, v
            for h in (h0, h1):
                rows.append(w_qkv[grp * C + h * Dh: grp * C + (h + 1) * Dh])
        wqkvT_c = np.ascontiguousarray(np.concatenate(rows, 0).T).astype(dt)
        wpT_c = np.ascontiguousarray(
            w_proj[:, 2 * Dh * c: 2 * Dh * (c + 1)].T).astype(dt)
        in_maps.append({"xT": xT, "wqkvT": wqkvT_c, "wpT": wpT_c})
    return in_maps


def kernel(x, w_qkv, w_proj, b_proj, _trace=False, _tmpdir=None):
    from concourse import bass_utils

    if "nc" not in _CACHE:
        _CACHE["nc"] = _build_program()
    nc = _CACHE["nc"]

    in_maps = _shard_inputs(
        np.asarray(x, np.float32),
        np.asarray(w_qkv, np.float32),
        np.asarray(w_proj, np.float32),
    )
    res = bass_utils.run_bass_kernel_spmd(
        nc, in_maps, core_ids=list(range(NCORES)),
        trace=_trace, tmpdir=_tmpdir,
    )
    total = res.results[0]["out"].astype(np.float32)
    for c in range(1, NCORES):
        total += res.results[c]["out"].astype(np.float32)
    total += np.asarray(b_proj, np.float32)[None, :]
    out = total.reshape(B, N, C)
    if _trace:
        return out, res
    return out
